# revision 13
# baseline (speedup 1.0000x reference)
"""Trainium2 Bass kernel for nn_BatchPSCN (gnn_message_passing).

Reference computation (per graph b of 128):
    x  = node_feat[recep[b] + b*4000]            # gather [4000, 128]
    y1 = relu(conv1d(x.T, conv1_w, stride=10))   # [32, 400]
    y2 = relu(conv1x1(y1, conv2_w))              # [32, 400]
    out[b] = sum(y2 * fc_w.reshape(32, 400)) + fc_b

Sharding: data-parallel over the 128 graphs, 16 graphs per NeuronCore on 8
cores; weights replicated; the gather is graph-local on each core.

Design notes (HW-measured on trn2):
  * node_feat is cast to fp16 on the host: halves the gather traffic; final
    absmax-relative error ~4e-4.
  * The gather uses the SWDGE dma_gather (natural row layout).  Its
    throughput is bound by Q7 descriptor generation (~8 ns/row/queue);
    queue q is served by GPSIMD core pair (2q, 2q+1), so striping chunks
    across all 4 SWDGE queues gives ~4x gather throughput.  (The xbar
    transpose-mode gather corrupts data when >1 queue runs concurrently —
    shared xbar state — so transposes are done on the tensor engine
    instead, fully hidden under the DMA.)
  * Alternatives measured and rejected this session: pool-engine
    InstIndirectCopy (~27 ns/idx, serial), indirect_dma_start (~1.4 us
    per 128-row instruction with distinct indices — it is SWDGE-backed
    and serializes on one dynamic queue), single_packet=True (wedges the
    device), 2-queue SWDGE (exactly 2x slower: generation-bound).  The
    4-queue SWDGE descriptor rate IS the kernel floor; compute (PE
    transposes + convs, ~68 us) hides fully under the ~124 us gather.
  * Gather positions are pre-permuted k-major on the host so each conv tap
    k reads a contiguous 400-column slice; conv1 is then 10 accumulating
    matmuls with the contraction over d=128 partitions.
  * Negative-index trimming of the 96 pad rows per graph does NOT help:
    the ucode pushes a dummy descriptor for every lane of a partial
    128-idx block, so only whole trimmed 128-blocks would save pushes
    (pad run is 96 < 128).  Confirmed neutral-to-worse on HW.
  * Four graphs share one gather group (gpg=4, 4096-row chunks): the
    serial per-instruction idx-load phase amortizes over bigger chunks —
    gather-only drops from ~119 us (gpg=2) to ~97 us (HW-measured, paired
    slope).  gpg=4/split=2 (8192-row chunks) is WORSE (~145 us: only two
    queues active per group), and gpg=8 wedges the device.  Group-slot
    indices are pre-offset by slot*4000 on the host.  The descriptor-ring
    scratch is halved to 32K/partition so the bigger tiles fit in SBUF.
  * A tiny warmup gather per queue absorbs a rare first-use race in the
    SWDGE queue rings; kernel() additionally runs the NEFF twice and
    falls back to a third run + median vote if the two disagree.
"""

import numpy as np

BS, N, D = 128, 4000, 128
NB, SEQ = 10, 400
C1, C2 = 32, 32
NCORES = 8
GPC = BS // NCORES          # graphs per core
NPAD = 4096                 # gather positions per graph (k-major + pad)
IDXC = NPAD // 16           # int16 index columns per graph (16-partition wrap)

GPG = 4                     # graphs per gather group
SPLIT = 4                   # gather chunks per group (striped over queues)
QUEUES = 4                  # SWDGE queues
SCRATCH = 32768             # SWDGE descriptor-ring carveout bytes (per partition;
                            # halved from 64K to fit gpg=4 tiles in SBUF)
XT_BUFS = 3
T_BUFS = 2

_PROGRAM = None


def _build_program(nrep: int = 1):
    import concourse.bacc as bacc
    import concourse.mybir as mybir
    import concourse.tile as tile

    f16, f32, i16 = mybir.dt.float16, mybir.dt.float32, mybir.dt.int16
    AF = mybir.ActivationFunctionType
    ALU = mybir.AluOpType

    ngroups = GPC // GPG
    gnpad = GPG * NPAD
    chunk = gnpad // SPLIT

    nc = bacc.Bacc(
        "TRN2",
        debug=False,
        num_devices=NCORES,
        dynamic_dma_scratch_size=SCRATCH,
        num_swdge_queues=QUEUES,
    )

    feat = nc.dram_tensor("feat", [GPC * N, D], f16, kind="ExternalInput")
    idxp = nc.dram_tensor("idxp", [128, GPC * IDXC], i16, kind="ExternalInput")
    w1 = nc.dram_tensor("w1", [D, NB * C1], f16, kind="ExternalInput")
    w2 = nc.dram_tensor("w2", [C1, C2], f16, kind="ExternalInput")
    fcw = nc.dram_tensor("fcw", [C2, SEQ], f32, kind="ExternalInput")
    b1 = nc.dram_tensor("b1", [C1, 1], f32, kind="ExternalInput")
    b2 = nc.dram_tensor("b2", [C2, 1], f32, kind="ExternalInput")
    fcb = nc.dram_tensor("fcb", [1, 1], f32, kind="ExternalInput")
    ident = nc.dram_tensor("ident", [128, 128], f16, kind="ExternalInput")
    out = nc.dram_tensor("out", [1, GPC], f32, kind="ExternalOutput")

    with tile.TileContext(nc) as tc:
        with (
            tc.tile_pool(name="const", bufs=1) as const,
            tc.tile_pool(name="xt", bufs=XT_BUFS) as xt_pool,
            tc.tile_pool(name="xT", bufs=2) as xT_pool,
            tc.tile_pool(name="y", bufs=2) as ypool,
            tc.tile_pool(name="ptr", bufs=T_BUFS, space="PSUM") as ptrans,
            tc.tile_pool(name="psum1", bufs=2, space="PSUM") as ppool1,
            tc.tile_pool(name="psum2", bufs=1, space="PSUM") as ppool2,
            tc.tile_pool(name="psumo", bufs=1, space="PSUM") as ppoolo,
        ):
            idxp_t = const.tile([128, GPC * IDXC], i16)
            nc.sync.dma_start(idxp_t[:], idxp[:])
            w1_t = const.tile([D, NB * C1], f16)
            nc.sync.dma_start(w1_t[:], w1[:])
            w2_t = const.tile([C1, C2], f16)
            nc.sync.dma_start(w2_t[:], w2[:])
            fcw_t = const.tile([C2, SEQ], f32)
            nc.sync.dma_start(fcw_t[:], fcw[:])
            b1_t = const.tile([C1, 1], f32)
            nc.sync.dma_start(b1_t[:], b1[:])
            b2_t = const.tile([C2, 1], f32)
            nc.sync.dma_start(b2_t[:], b2[:])
            fcb_t = const.tile([1, 1], f32)
            nc.sync.dma_start(fcb_t[:], fcb[:])
            ident_t = const.tile([128, 128], f16)
            nc.sync.dma_start(ident_t[:], ident[:])
            ones = const.tile([C2, 1], f32)
            nc.vector.memset(ones[:], 1.0)
            R = const.tile([C2, GPC], f32)

            # warmup: absorb first-use flakiness of each SWDGE queue ring
            idx0 = const.tile([128, 8], i16)
            nc.vector.memset(idx0[:].bitcast(f16), 0.0)
            wscr = const.tile([128, 1, 128], f16, tag="wscr")
            for q in range(QUEUES):
                nc.gpsimd.dma_gather(
                    wscr[:],
                    feat[0:N, :],
                    idx0[:],
                    128,
                    128,
                    D,
                    transpose=False,
                    single_packet=False,
                    queue_num=q,
                )

            qrr = [0]

            def next_queue():
                q = qrr[0]
                qrr[0] = (qrr[0] + 1) % QUEUES
                return q

            for rep in range(nrep):
              for grp in range(ngroups):
                xt = xt_pool.tile([128, gnpad // 128, 128], f16)
                base = feat[grp * GPG * N : (grp * GPG + GPG) * N, :]
                for s in range(SPLIT):
                    nc.gpsimd.dma_gather(
                        xt[:, s * (chunk // 128) : (s + 1) * (chunk // 128), :],
                        base,
                        idxp_t[
                            :,
                            grp * GPG * IDXC
                            + s * chunk // 16 : grp * GPG * IDXC
                            + (s + 1) * chunk // 16,
                        ],
                        chunk,
                        chunk,
                        D,
                        transpose=False,
                        single_packet=False,
                        queue_num=next_queue(),
                    )
                # transpose [i, d] -> [d, i] via PE, 4 blocks per PSUM tile
                xT = xT_pool.tile([128, gnpad], f16)
                for blk in range(gnpad // 512):
                    pt = ptrans.tile([128, 512], f16)
                    for j in range(4):
                        c = blk * 4 + j
                        nc.tensor.transpose(
                            pt[:, j * 128 : (j + 1) * 128], xt[:, c, :], ident_t[:]
                        )
                    if blk % 2 == 0:
                        nc.scalar.copy(xT[:, blk * 512 : (blk + 1) * 512], pt[:])
                    else:
                        nc.vector.tensor_copy(
                            xT[:, blk * 512 : (blk + 1) * 512], pt[:]
                        )
                for gl in range(GPG):
                    g = grp * GPG + gl
                    base_col = gl * NPAD
                    psum1 = ppool1.tile([C1, SEQ], f32)
                    for k in range(NB):
                        nc.tensor.matmul(
                            psum1[:],
                            w1_t[:, k * C1 : (k + 1) * C1],
                            xT[:, base_col + k * SEQ : base_col + (k + 1) * SEQ],
                            start=(k == 0),
                            stop=(k == NB - 1),
                        )
                    y1 = ypool.tile([C1, SEQ], f16, tag="y1")
                    nc.scalar.activation(y1[:], psum1[:], AF.Relu, bias=b1_t[:])
                    psum2 = ppool2.tile([C2, SEQ], f32)
                    nc.tensor.matmul(psum2[:], w2_t[:], y1[:], start=True, stop=True)
                    y2 = ypool.tile([C2, SEQ], f32, tag="y2")
                    nc.scalar.activation(y2[:], psum2[:], AF.Relu, bias=b2_t[:])
                    prod = ypool.tile([C2, SEQ], f32, tag="prod")
                    nc.vector.tensor_mul(prod[:], y2[:], fcw_t[:])
                    nc.vector.tensor_reduce(
                        R[:, g : g + 1], prod[:], mybir.AxisListType.X, ALU.add
                    )

              psum_o = ppoolo.tile([1, GPC], f32)
              nc.tensor.matmul(psum_o[:], ones[:], R[:], start=True, stop=True)
              out_t = const.tile([1, GPC], f32, tag="out_t")
              nc.scalar.activation(out_t[:], psum_o[:], AF.Identity, bias=fcb_t[:])
              nc.sync.dma_start(out[:], out_t[:])

    nc.compile()
    return nc


def get_program():
    global _PROGRAM
    if _PROGRAM is None:
        _PROGRAM = _build_program()
    return _PROGRAM


def make_in_maps(recep, node_feat, conv1_w, conv1_b, conv2_w, conv2_b, fc_w, fc_b):
    recep = np.asarray(recep)
    node_feat = np.asarray(node_feat)

    feat16 = np.asarray(node_feat, dtype=np.float16)  # [BS*N, D]

    # k-major permutation: gather position k*SEQ+s holds node recep[b, s*NB+k].
    # Pad slots keep index 0: the gather ucode pushes a (dummy) descriptor for
    # every lane of a partial 128-block anyway, so negative-index trimming of
    # the 96-row graph tails saves nothing (measured + ucode-confirmed).
    perm = recep.reshape(BS, SEQ, NB).transpose(0, 2, 1).reshape(BS, N)
    permp = np.concatenate(
        [perm, np.zeros((BS, NPAD - N), perm.dtype)], axis=1
    )  # [BS, NPAD]
    # wrap across 16 partitions (idx i -> [i % 16, i // 16]), replicate x8
    wrapped = permp.reshape(BS, IDXC, 16).transpose(0, 2, 1)  # [BS, 16, IDXC]
    idx_all = np.tile(wrapped, (1, 8, 1))  # [BS, 128, IDXC]

    identh = np.eye(128, dtype=np.float16)
    w1h = (
        np.asarray(conv1_w).transpose(1, 2, 0).reshape(D, NB * C1).astype(np.float16)
    )
    w2h = np.asarray(conv2_w)[:, :, 0].T.astype(np.float16)  # [C1, C2]
    fcwh = np.asarray(fc_w).reshape(C2, SEQ).astype(np.float32)
    b1h = np.asarray(conv1_b).reshape(C1, 1).astype(np.float32)
    b2h = np.asarray(conv2_b).reshape(C2, 1).astype(np.float32)
    fcbh = np.asarray(fc_b).reshape(1, 1).astype(np.float32)

    in_maps = []
    for c in range(NCORES):
        g0 = c * GPC
        # grouped layout: graph gl indexes into its slot of the GPG-graph slab
        idxp_core = np.concatenate(
            [idx_all[g0 + gl] + (gl % GPG) * N for gl in range(GPC)], axis=1
        ).astype(np.int16)
        in_maps.append(
            {
                "feat": np.ascontiguousarray(feat16[g0 * N : (g0 + GPC) * N]),
                "idxp": np.ascontiguousarray(idxp_core),
                "w1": w1h,
                "w2": w2h,
                "fcw": fcwh,
                "b1": b1h,
                "b2": b2h,
                "fcb": fcbh,
                "ident": identh,
            }
        )
    return in_maps


def _run_once(nc, in_maps):
    from concourse.bass_utils import run_bass_kernel_spmd

    res = run_bass_kernel_spmd(nc, in_maps, list(range(NCORES))).results
    out = np.concatenate(
        [np.asarray(res[c]["out"]).reshape(GPC) for c in range(NCORES)]
    )
    return out[:, None].astype(np.float32)


def kernel(recep, node_feat, conv1_w, conv1_b, conv2_w, conv2_b, fc_w, fc_b):
    nc = get_program()
    in_maps = make_in_maps(
        recep, node_feat, conv1_w, conv1_b, conv2_w, conv2_b, fc_w, fc_b
    )
    # The kernel is deterministic; two clean runs agree bit-exactly.  A rare
    # SWDGE-ring first-use race can corrupt a run, so vote.
    r1 = _run_once(nc, in_maps)
    r2 = _run_once(nc, in_maps)
    if np.array_equal(r1, r2):
        return r1
    r3 = _run_once(nc, in_maps)
    return np.median(np.stack([r1, r2, r3]), axis=0).astype(np.float32)

